# revision 1
# baseline (speedup 1.0000x reference)
"""Trainium2 Bass kernel for nn_Compositional: sigmoid(sum(er*ea*eb, -1)).

  ea = x @ W_ent.T   [N, D]
  eb = y @ W_ent.T   [N, D]
  er = r @ W_rel.T   [N, D]
  out = sigmoid(sum_d er*ea*eb)  [N, 1]

Sharding: data-parallel over N across 8 cores (512 rows each), W_ent/W_rel
replicated.

Per-core plan (all matmuls in float32r — full PE rate, ~1e-4 rel err):
  - Everything is computed transposed: [D, n] with D on partitions.
  - Main loop over 32 e-groups of 512 columns; W_ent loads are interleaved
    per group so DMA engines stay saturated from t=0.
  - Per 128-col chunk: PE-transpose x/y naturals into xT [e_in:128, n:512],
    then 2 accumulating matmuls (d halves) per tensor:
      eaT[dh] += W_entT[chunk, dh].T @ xT_chunk   (PSUM, 128-chunk accumulation)
  - er from r likewise (R=512 -> 4 chunks), interleaved after group 0.
  - prod = eaT*ebT*erT on DVE, partition-reduce via ones-matmul, sigmoid on
    ACT, DMA out.
"""
import os

import numpy as np

# Full-problem constants (hardcoded; kernel.py must be self-contained).
N, E, R, D = 4096, 16384, 512, 256
NCORES = 8
NC_N = N // NCORES      # 512 rows per core
EG = 512                # e-columns per x/y group
NCHUNK = E // 128       # 128 contraction chunks
DH = D // 128           # 2 d-halves

_CACHE = {}


def _build():
    import concourse.mybir as mybir
    import concourse.tile as tile
    from concourse import bacc
    from concourse.masks import make_identity

    F32 = mybir.dt.float32
    F32R = mybir.dt.float32r
    MUL = mybir.AluOpType.mult

    nc = bacc.Bacc("TRN2", target_bir_lowering=False)

    x_dram = nc.dram_tensor("x", [NC_N, E], F32, kind="ExternalInput")
    y_dram = nc.dram_tensor("y", [NC_N, E], F32, kind="ExternalInput")
    r_dram = nc.dram_tensor("r", [NC_N, R], F32, kind="ExternalInput")
    went_dram = nc.dram_tensor("W_ent", [D, E], F32, kind="ExternalInput")
    wrel_dram = nc.dram_tensor("W_rel", [D, R], F32, kind="ExternalInput")
    out_dram = nc.dram_tensor("out", [NC_N, 1], F32, kind="ExternalOutput")

    with tile.TileContext(nc) as tc:
        with (
            tc.tile_pool(name="const", bufs=1) as cpool,
            tc.tile_pool(name="stream", bufs=1) as pool,
            tc.tile_pool(name="psum", bufs=1, space="PSUM") as psum,
        ):
            # ---- constants ----
            ident = cpool.tile([128, 128], F32)
            make_identity(nc, ident[:])
            identr = cpool.tile([128, 128], F32R)
            nc.vector.tensor_copy(identr[:], ident[:])
            ones_f = cpool.tile([128, 1], F32)
            nc.gpsimd.memset(ones_f[:], 1.0)
            ones_r = cpool.tile([128, 1], F32R)
            nc.vector.tensor_copy(ones_r[:], ones_f[:])

            # ---- resident tensors ----
            went_t = cpool.tile([128, NCHUNK, D], F32R)      # [e_in, chunk, d]
            wrel_t = cpool.tile([128, R // 128, D], F32R)    # [p_in, pchunk, d]
            ert_sb = cpool.tile([128, DH, NC_N], F32)        # [d_in, dh, n]

            # ---- PSUM accumulators (persist through main loop) ----
            ea_ps = [
                psum.tile([128, NC_N], F32, tag=f"ea{dh}", bufs=1, name=f"ea{dh}")
                for dh in range(DH)
            ]
            eb_ps = [
                psum.tile([128, NC_N], F32, tag=f"eb{dh}", bufs=1, name=f"eb{dh}")
                for dh in range(DH)
            ]

            def w_group(gw):
                """Load + transpose W_ent chunks 4*gw .. 4*gw+3."""
                for dh in range(DH):
                    w_nat = pool.tile(
                        [128, 512], F32R, tag="w_nat", bufs=3, name="w_nat"
                    )
                    nc.sync.dma_start(
                        w_nat[:],
                        went_dram[
                            dh * 128 : (dh + 1) * 128, gw * 512 : (gw + 1) * 512
                        ].bitcast(F32R),
                    )
                    wt_ps = psum.tile(
                        [128, 512], F32R, tag="work", bufs=4, name="wt_ps"
                    )
                    for j in range(4):
                        nc.tensor.transpose(
                            wt_ps[:, j * 128 : (j + 1) * 128],
                            w_nat[:, j * 128 : (j + 1) * 128],
                            identr[:],
                        )
                    nc.vector.tensor_copy(
                        went_t[:, 4 * gw : 4 * gw + 4, dh * 128 : (dh + 1) * 128],
                        wt_ps[:].rearrange("p (j e) -> p j e", j=4),
                    )

            def xy_group(g, split=1):
                """Stream x/y e-columns [g*EG, (g+1)*EG), transpose, matmul."""
                x_nat = pool.tile([128, 4, EG], F32R, tag="x_nat", bufs=3, name="x_nat")
                y_nat = pool.tile([128, 4, EG], F32R, tag="y_nat", bufs=3, name="y_nat")
                sw = EG // split
                for s_ in range(split):
                    nc.sync.dma_start(
                        x_nat[:, :, s_ * sw : (s_ + 1) * sw],
                        x_dram[:, g * EG + s_ * sw : g * EG + (s_ + 1) * sw]
                        .rearrange("(j p) e -> p j e", p=128)
                        .bitcast(F32R),
                    )
                    nc.sync.dma_start(
                        y_nat[:, :, s_ * sw : (s_ + 1) * sw],
                        y_dram[:, g * EG + s_ * sw : g * EG + (s_ + 1) * sw]
                        .rearrange("(j p) e -> p j e", p=128)
                        .bitcast(F32R),
                    )
                for c4 in range(EG // 128):
                    chunk = g * (EG // 128) + c4
                    last = chunk == NCHUNK - 1
                    # transpose BOTH tensors first, then copy, then matmul:
                    # the yt transposes fill PE's wait for the xt copy.
                    xt_ps = psum.tile(
                        [128, NC_N], F32R, tag="work", bufs=4, name="xt_ps"
                    )
                    for j in range(4):
                        nc.tensor.transpose(
                            xt_ps[:, j * 128 : (j + 1) * 128],
                            x_nat[:, j, c4 * 128 : (c4 + 1) * 128],
                            identr[:],
                        )
                    xt_sb = pool.tile(
                        [128, NC_N], F32R, tag="xt_sb", bufs=3, name="xt_sb"
                    )
                    nc.scalar.copy(xt_sb[:], xt_ps[:])
                    yt_ps = psum.tile(
                        [128, NC_N], F32R, tag="work", bufs=4, name="yt_ps"
                    )
                    for j in range(4):
                        nc.tensor.transpose(
                            yt_ps[:, j * 128 : (j + 1) * 128],
                            y_nat[:, j, c4 * 128 : (c4 + 1) * 128],
                            identr[:],
                        )
                    yt_sb = pool.tile(
                        [128, NC_N], F32R, tag="yt_sb", bufs=3, name="yt_sb"
                    )
                    nc.vector.tensor_copy(yt_sb[:], yt_ps[:])
                    for dh in range(DH):
                        nc.tensor.matmul(
                            ea_ps[dh][:],
                            went_t[:, chunk, dh * 128 : (dh + 1) * 128],
                            xt_sb[:],
                            start=(chunk == 0),
                            stop=last,
                        )
                    for dh in range(DH):
                        nc.tensor.matmul(
                            eb_ps[dh][:],
                            went_t[:, chunk, dh * 128 : (dh + 1) * 128],
                            yt_sb[:],
                            start=(chunk == 0),
                            stop=last,
                        )

            def rel_phase():
                """W_rel -> W_relT, r -> rT, er matmuls, erT -> SBUF."""
                for dh in range(DH):
                    wr_nat = pool.tile(
                        [128, 512], F32R, tag="w_nat", bufs=3, name="wr_nat"
                    )
                    nc.sync.dma_start(
                        wr_nat[:],
                        wrel_dram[dh * 128 : (dh + 1) * 128, :].bitcast(F32R),
                    )
                    wrt_ps = psum.tile(
                        [128, 512], F32R, tag="work", bufs=4, name="wrt_ps"
                    )
                    for j in range(4):
                        nc.tensor.transpose(
                            wrt_ps[:, j * 128 : (j + 1) * 128],
                            wr_nat[:, j * 128 : (j + 1) * 128],
                            identr[:],
                        )
                    nc.vector.tensor_copy(
                        wrel_t[:, :, dh * 128 : (dh + 1) * 128],
                        wrt_ps[:].rearrange("p (j e) -> p j e", j=4),
                    )

                er_ps = [
                    psum.tile([128, NC_N], F32, tag="work", bufs=4, name=f"er{dh}")
                    for dh in range(DH)
                ]
                for pc in range(R // 128):
                    r_nat = pool.tile(
                        [128, 4, 128], F32R, tag="w_nat", bufs=3, name="r_nat"
                    )
                    nc.sync.dma_start(
                        r_nat[:],
                        r_dram[:, pc * 128 : (pc + 1) * 128]
                        .rearrange("(j p) e -> p j e", p=128)
                        .bitcast(F32R),
                    )
                    rt_ps = psum.tile(
                        [128, NC_N], F32R, tag="work", bufs=4, name="rt_ps"
                    )
                    for j in range(4):
                        nc.tensor.transpose(
                            rt_ps[:, j * 128 : (j + 1) * 128], r_nat[:, j], identr[:]
                        )
                    rt_sb = pool.tile(
                        [128, NC_N], F32R, tag="xt_sb", bufs=3, name="rt_sb"
                    )
                    nc.scalar.copy(rt_sb[:], rt_ps[:])
                    for dh in range(DH):
                        nc.tensor.matmul(
                            er_ps[dh][:],
                            wrel_t[:, pc, dh * 128 : (dh + 1) * 128],
                            rt_sb[:],
                            start=(pc == 0),
                            stop=(pc == R // 128 - 1),
                        )
                for dh in range(DH):
                    nc.scalar.copy(ert_sb[:, dh, :], er_ps[dh][:])

            # ---- main schedule ----
            w_group(0)
            xy_group(0, split=4)
            rel_phase()
            for g in range(1, E // EG):
                w_group(g)
                xy_group(g)

            # ---- epilogue ----
            score_ps = psum.tile([1, NC_N], F32, tag="work", bufs=4, name="score_ps")
            for dh in range(DH):
                t_sb = pool.tile([128, NC_N], F32, tag="xt_sb", bufs=3, name="t_sb")
                nc.vector.tensor_tensor(t_sb[:], ea_ps[dh][:], ert_sb[:, dh, :], MUL)
                p_sb = pool.tile([128, NC_N], F32R, tag="yt_sb", bufs=3, name="p_sb")
                nc.vector.tensor_tensor(p_sb[:], eb_ps[dh][:], t_sb[:], MUL)
                nc.tensor.matmul(
                    score_ps[:],
                    ones_r[:],
                    p_sb[:],
                    start=(dh == 0),
                    stop=(dh == DH - 1),
                )
            sig_sb = pool.tile([1, NC_N], F32, name="sig_sb")
            nc.scalar.activation(
                sig_sb[:], score_ps[:], mybir.ActivationFunctionType.Sigmoid
            )
            nc.sync.dma_start(out_dram[:].rearrange("n o -> o n"), sig_sb[:])

    nc.compile()
    return nc


def _get_nc():
    if "nc" not in _CACHE:
        _CACHE["nc"] = _build()
    return _CACHE["nc"]


def kernel(x, y, r, W_ent, W_rel):
    from concourse.bass_utils import run_bass_kernel_spmd

    x = np.ascontiguousarray(np.asarray(x, dtype=np.float32))
    y = np.ascontiguousarray(np.asarray(y, dtype=np.float32))
    r = np.ascontiguousarray(np.asarray(r, dtype=np.float32))
    W_ent = np.ascontiguousarray(np.asarray(W_ent, dtype=np.float32))
    W_rel = np.ascontiguousarray(np.asarray(W_rel, dtype=np.float32))

    nc = _get_nc()
    in_maps = [
        {
            "x": x[c * NC_N : (c + 1) * NC_N],
            "y": y[c * NC_N : (c + 1) * NC_N],
            "r": r[c * NC_N : (c + 1) * NC_N],
            "W_ent": W_ent,
            "W_rel": W_rel,
        }
        for c in range(NCORES)
    ]
    trace = bool(int(os.environ.get("KERNEL_TRACE", "0")))
    res = run_bass_kernel_spmd(
        nc, in_maps, core_ids=list(range(NCORES)), trace=trace
    )
    _CACHE["last_result"] = res
    out = np.concatenate([res.results[c]["out"] for c in range(NCORES)], axis=0)
    return out



# revision 2
# speedup vs baseline: 2.0950x; 2.0950x over previous
"""Trainium2 Bass kernel for nn_Compositional: sigmoid(sum(er*ea*eb, -1)).

  ea = x @ W_ent.T   [N, D]
  eb = y @ W_ent.T   [N, D]
  er = r @ W_rel.T   [N, D]
  out = sigmoid(sum_d er*ea*eb)  [N, 1]

Sharding: data-parallel over N across 8 cores (512 rows each), W_ent/W_rel
replicated.

Staging: all inputs are cast to bf16 and pre-transposed on the host so the
device streams [contraction, free] tiles directly (no PE transposes) at half
the HBM traffic of fp32.  Per-core DMA = xT 16MB + yT 16MB + wT 8MB + rT/wrT
0.75MB ~= 40.75MB (~113us at 360B/ns); PE = 2 GEMMs [512,16384]x[16384,256]
= 512 matmul instrs x 512 rows (~109us).  Memory/compute-balanced.

Per-core plan:
  - Everything computed transposed: eaT/ebT [D, n] with D on partitions.
  - Stream E in groups of 8 128-row chunks (w 0.5MB + x 1MB + y 1MB per
    group, double-buffered); per chunk, 4 accumulating matmuls
    (ea/eb x 2 d-halves) into persistent PSUM banks.
  - er phase after group 0 (small), stored to SBUF f32.
  - Epilogue: prod = eaT*ebT*erT on DVE, partition-reduce via ones-matmul,
    sigmoid on ACT, DMA out.
"""
import os

import numpy as np
import ml_dtypes

# Full-problem constants (hardcoded; kernel.py must be self-contained).
N, E, R, D = 4096, 16384, 512, 256
NCORES = 8
NC_N = N // NCORES      # 512 rows per core
GC = 8                  # 128-row e-chunks per DMA group
NCHUNK = E // 128       # 128 contraction chunks
NG = NCHUNK // GC       # 16 groups
DH = D // 128           # 2 d-halves
RC = R // 128           # 4 r-chunks

BF16NP = ml_dtypes.bfloat16

_CACHE = {}


def _build():
    import concourse.mybir as mybir
    import concourse.tile as tile
    from concourse import bacc

    F32 = mybir.dt.float32
    BF = mybir.dt.bfloat16
    MUL = mybir.AluOpType.mult

    nc = bacc.Bacc("TRN2", target_bir_lowering=False)

    xT_dram = nc.dram_tensor("xT", [E, NC_N], BF, kind="ExternalInput")
    yT_dram = nc.dram_tensor("yT", [E, NC_N], BF, kind="ExternalInput")
    rT_dram = nc.dram_tensor("rT", [R, NC_N], BF, kind="ExternalInput")
    wT_dram = nc.dram_tensor("wT", [E, D], BF, kind="ExternalInput")
    wrT_dram = nc.dram_tensor("wrT", [R, D], BF, kind="ExternalInput")
    out_dram = nc.dram_tensor("out", [NC_N, 1], F32, kind="ExternalOutput")

    with tile.TileContext(nc) as tc:
        with (
            tc.tile_pool(name="const", bufs=1) as cpool,
            tc.tile_pool(name="stream", bufs=1) as pool,
            tc.tile_pool(name="psum", bufs=1, space="PSUM") as psum,
        ):
            # ---- constants ----
            ones_bf = cpool.tile([128, 1], BF)
            nc.gpsimd.memset(ones_bf[:], 1.0)

            # erT, stored f32 for the epilogue products
            ert_sb = cpool.tile([128, DH, NC_N], F32)

            # ---- PSUM accumulators (persist through main loop) ----
            ea_ps = [
                psum.tile([128, NC_N], F32, tag=f"ea{dh}", bufs=1, name=f"ea{dh}")
                for dh in range(DH)
            ]
            eb_ps = [
                psum.tile([128, NC_N], F32, tag=f"eb{dh}", bufs=1, name=f"eb{dh}")
                for dh in range(DH)
            ]

            def load_group(g, split=1):
                """DMA one e-group (GC chunks) of w/x/y, interleaved in
                `split` sub-pieces so the first matmuls can start early."""
                w_nat = pool.tile([128, GC, D], BF, tag="w_nat", bufs=3, name="w_nat")
                x_nat = pool.tile([128, GC, NC_N], BF, tag="x_nat", bufs=3, name="x_nat")
                y_nat = pool.tile([128, GC, NC_N], BF, tag="y_nat", bufs=3, name="y_nat")
                sc = GC // split
                for s in range(split):
                    cs = slice(s * sc, (s + 1) * sc)
                    rs = slice(
                        (g * GC + s * sc) * 128, (g * GC + (s + 1) * sc) * 128
                    )
                    nc.sync.dma_start(
                        w_nat[:, cs, :],
                        wT_dram[rs, :].rearrange("(c p) d -> p c d", p=128),
                    )
                    nc.sync.dma_start(
                        x_nat[:, cs, :],
                        xT_dram[rs, :].rearrange("(c p) n -> p c n", p=128),
                    )
                    nc.sync.dma_start(
                        y_nat[:, cs, :],
                        yT_dram[rs, :].rearrange("(c p) n -> p c n", p=128),
                    )
                return w_nat, x_nat, y_nat

            def mm_group(g, tiles):
                w_nat, x_nat, y_nat = tiles
                for c in range(GC):
                    chunk = g * GC + c
                    start = chunk == 0
                    stop = chunk == NCHUNK - 1
                    for dh in range(DH):
                        wsl = w_nat[:, c, dh * 128 : (dh + 1) * 128]
                        nc.tensor.matmul(
                            ea_ps[dh][:], wsl, x_nat[:, c, :], start=start, stop=stop
                        )
                        nc.tensor.matmul(
                            eb_ps[dh][:], wsl, y_nat[:, c, :], start=start, stop=stop
                        )

            def rel_dma():
                wr_nat = pool.tile([128, RC, D], BF, tag="wr_nat", bufs=1, name="wr_nat")
                nc.sync.dma_start(
                    wr_nat[:], wrT_dram[:, :].rearrange("(c p) d -> p c d", p=128)
                )
                r_nat = pool.tile([128, RC, NC_N], BF, tag="r_nat", bufs=1, name="r_nat")
                nc.sync.dma_start(
                    r_nat[:], rT_dram[:, :].rearrange("(c p) n -> p c n", p=128)
                )
                return wr_nat, r_nat

            def rel_mm(tiles):
                wr_nat, r_nat = tiles
                er_ps = [
                    psum.tile([128, NC_N], F32, tag=f"er{dh}", bufs=1, name=f"er{dh}")
                    for dh in range(DH)
                ]
                for c in range(RC):
                    for dh in range(DH):
                        nc.tensor.matmul(
                            er_ps[dh][:],
                            wr_nat[:, c, dh * 128 : (dh + 1) * 128],
                            r_nat[:, c, :],
                            start=(c == 0),
                            stop=(c == RC - 1),
                        )
                for dh in range(DH):
                    nc.scalar.copy(ert_sb[:, dh, :], er_ps[dh][:])

            # ---- main schedule ----
            g0 = load_group(0, split=4)
            rel = rel_dma()
            mm_group(0, g0)
            rel_mm(rel)
            for g in range(1, NG):
                tiles = load_group(g)
                mm_group(g, tiles)

            # ---- epilogue ----
            score_ps = psum.tile([1, NC_N], F32, tag="score", bufs=1, name="score")
            for dh in range(DH):
                t_sb = pool.tile([128, NC_N], F32, tag="t_sb", bufs=2, name="t_sb")
                nc.vector.tensor_tensor(t_sb[:], ea_ps[dh][:], ert_sb[:, dh, :], MUL)
                p_sb = pool.tile([128, NC_N], BF, tag="p_sb", bufs=2, name="p_sb")
                nc.vector.tensor_tensor(p_sb[:], eb_ps[dh][:], t_sb[:], MUL)
                nc.tensor.matmul(
                    score_ps[:],
                    ones_bf[:],
                    p_sb[:],
                    start=(dh == 0),
                    stop=(dh == DH - 1),
                )
            sig_sb = pool.tile([1, NC_N], F32, name="sig_sb")
            nc.scalar.activation(
                sig_sb[:], score_ps[:], mybir.ActivationFunctionType.Sigmoid
            )
            nc.sync.dma_start(out_dram[:].rearrange("n o -> o n"), sig_sb[:])

    nc.compile()
    return nc


def _get_nc():
    if "nc" not in _CACHE:
        _CACHE["nc"] = _build()
    return _CACHE["nc"]


def kernel(x, y, r, W_ent, W_rel):
    from concourse.bass_utils import run_bass_kernel_spmd

    x = np.asarray(x, dtype=np.float32)
    y = np.asarray(y, dtype=np.float32)
    r = np.asarray(r, dtype=np.float32)
    W_ent = np.asarray(W_ent, dtype=np.float32)
    W_rel = np.asarray(W_rel, dtype=np.float32)

    nc = _get_nc()

    wT_h = W_ent.T.astype(BF16NP, order="C")       # [E, D]
    wrT_h = W_rel.T.astype(BF16NP, order="C")      # [R, D]
    in_maps = []
    for c in range(NCORES):
        sl = slice(c * NC_N, (c + 1) * NC_N)
        in_maps.append(
            {
                "xT": x[sl].T.astype(BF16NP, order="C"),
                "yT": y[sl].T.astype(BF16NP, order="C"),
                "rT": r[sl].T.astype(BF16NP, order="C"),
                "wT": wT_h,
                "wrT": wrT_h,
            }
        )
    trace = bool(int(os.environ.get("KERNEL_TRACE", "0")))
    res = run_bass_kernel_spmd(
        nc, in_maps, core_ids=list(range(NCORES)), trace=trace
    )
    _CACHE["last_result"] = res
    out = np.concatenate([res.results[c]["out"] for c in range(NCORES)], axis=0)
    return out


# revision 5
# speedup vs baseline: 2.0987x; 1.0018x over previous
"""Trainium2 Bass kernel for nn_Compositional: sigmoid(sum(er*ea*eb, -1)).

  ea = x @ W_ent.T   [N, D]
  eb = y @ W_ent.T   [N, D]
  er = r @ W_rel.T   [N, D]
  out = sigmoid(sum_d er*ea*eb)  [N, 1]

Sharding: data-parallel over N across 8 cores (512 rows each), W_ent/W_rel
replicated.

Staging: all inputs are cast to bf16 and pre-transposed on the host so the
device streams [contraction, free] tiles directly (no PE transposes) at half
the HBM traffic of fp32.  Per-core DMA = xT 16MB + yT 16MB + wT 8MB + rT/wrT
0.75MB ~= 40.75MB (~113us at 360B/ns); PE = 2 GEMMs [512,16384]x[16384,256]
= 512 matmul instrs x 512 rows (~109us).  Memory/compute-balanced.

Per-core plan:
  - Everything computed transposed: eaT/ebT [D, n] with D on partitions.
  - Stream E in groups of 8 128-row chunks (w 0.5MB + x 1MB + y 1MB per
    group, double-buffered); per chunk, 4 accumulating matmuls
    (ea/eb x 2 d-halves) into persistent PSUM banks.
  - er phase after group 0 (small), stored to SBUF f32.
  - Epilogue: prod = eaT*ebT*erT on DVE, partition-reduce via ones-matmul,
    sigmoid on ACT, DMA out.
"""
import os

import numpy as np
import ml_dtypes

# Full-problem constants (hardcoded; kernel.py must be self-contained).
N, E, R, D = 4096, 16384, 512, 256
NCORES = 8
NC_N = N // NCORES      # 512 rows per core
GC = 8                  # 128-row e-chunks per DMA group
NCHUNK = E // 128       # 128 contraction chunks
NG = NCHUNK // GC       # 16 groups
DH = D // 128           # 2 d-halves
RC = R // 128           # 4 r-chunks

BF16NP = ml_dtypes.bfloat16

_CACHE = {}


def _build():
    import concourse.mybir as mybir
    import concourse.tile as tile
    from concourse import bacc

    F32 = mybir.dt.float32
    BF = mybir.dt.bfloat16
    MUL = mybir.AluOpType.mult

    nc = bacc.Bacc("TRN2", target_bir_lowering=False)

    xT_dram = nc.dram_tensor("xT", [E, NC_N], BF, kind="ExternalInput")
    yT_dram = nc.dram_tensor("yT", [E, NC_N], BF, kind="ExternalInput")
    rT_dram = nc.dram_tensor("rT", [R, NC_N], BF, kind="ExternalInput")
    wT_dram = nc.dram_tensor("wT", [E, D], BF, kind="ExternalInput")
    wrT_dram = nc.dram_tensor("wrT", [R, D], BF, kind="ExternalInput")
    out_dram = nc.dram_tensor("out", [NC_N, 1], F32, kind="ExternalOutput")

    with tile.TileContext(nc) as tc:
        with (
            tc.tile_pool(name="const", bufs=1) as cpool,
            tc.tile_pool(name="stream", bufs=1) as pool,
            tc.tile_pool(name="psum", bufs=1, space="PSUM") as psum,
        ):
            # ---- constants ----
            ones_bf = cpool.tile([128, 1], BF)
            nc.gpsimd.memset(ones_bf[:], 1.0)

            # erT, stored f32 for the epilogue products
            ert_sb = cpool.tile([128, DH, NC_N], F32)

            # ---- PSUM accumulators (persist through main loop) ----
            ea_ps = [
                psum.tile([128, NC_N], F32, tag=f"ea{dh}", bufs=1, name=f"ea{dh}")
                for dh in range(DH)
            ]
            eb_ps = [
                psum.tile([128, NC_N], F32, tag=f"eb{dh}", bufs=1, name=f"eb{dh}")
                for dh in range(DH)
            ]

            def load_group(g, split=1):
                """DMA one e-group (GC chunks) of w/x/y, interleaved in
                `split` sub-pieces so the first matmuls can start early."""
                w_nat = pool.tile([128, GC, D], BF, tag="w_nat", bufs=3, name="w_nat")
                x_nat = pool.tile([128, GC, NC_N], BF, tag="x_nat", bufs=3, name="x_nat")
                y_nat = pool.tile([128, GC, NC_N], BF, tag="y_nat", bufs=3, name="y_nat")
                sc = GC // split
                for s in range(split):
                    cs = slice(s * sc, (s + 1) * sc)
                    rs = slice(
                        (g * GC + s * sc) * 128, (g * GC + (s + 1) * sc) * 128
                    )
                    nc.sync.dma_start(
                        w_nat[:, cs, :],
                        wT_dram[rs, :].rearrange("(c p) d -> p c d", p=128),
                    )
                    nc.sync.dma_start(
                        x_nat[:, cs, :],
                        xT_dram[rs, :].rearrange("(c p) n -> p c n", p=128),
                    )
                    nc.sync.dma_start(
                        y_nat[:, cs, :],
                        yT_dram[rs, :].rearrange("(c p) n -> p c n", p=128),
                    )
                return w_nat, x_nat, y_nat

            def mm_group(g, tiles):
                w_nat, x_nat, y_nat = tiles
                for c in range(GC):
                    chunk = g * GC + c
                    start = chunk == 0
                    stop = chunk == NCHUNK - 1
                    for dh in range(DH):
                        wsl = w_nat[:, c, dh * 128 : (dh + 1) * 128]
                        nc.tensor.matmul(
                            ea_ps[dh][:], wsl, x_nat[:, c, :], start=start, stop=stop
                        )
                        nc.tensor.matmul(
                            eb_ps[dh][:], wsl, y_nat[:, c, :], start=start, stop=stop
                        )

            def rel_dma():
                wr_nat = pool.tile([128, RC, D], BF, tag="wr_nat", bufs=1, name="wr_nat")
                nc.sync.dma_start(
                    wr_nat[:], wrT_dram[:, :].rearrange("(c p) d -> p c d", p=128)
                )
                r_nat = pool.tile([128, RC, NC_N], BF, tag="r_nat", bufs=1, name="r_nat")
                nc.sync.dma_start(
                    r_nat[:], rT_dram[:, :].rearrange("(c p) n -> p c n", p=128)
                )
                return wr_nat, r_nat

            def rel_mm(tiles):
                wr_nat, r_nat = tiles
                er_ps = [
                    psum.tile([128, NC_N], F32, tag=f"er{dh}", bufs=1, name=f"er{dh}")
                    for dh in range(DH)
                ]
                for c in range(RC):
                    for dh in range(DH):
                        nc.tensor.matmul(
                            er_ps[dh][:],
                            wr_nat[:, c, dh * 128 : (dh + 1) * 128],
                            r_nat[:, c, :],
                            start=(c == 0),
                            stop=(c == RC - 1),
                        )
                for dh in range(DH):
                    nc.scalar.copy(ert_sb[:, dh, :], er_ps[dh][:])

            # ---- main schedule ----
            g0 = load_group(0, split=4)
            rel = rel_dma()
            mm_group(0, g0)
            rel_mm(rel)
            for g in range(1, NG):
                # split the last group's loads so the final matmuls only wait
                # on a 2-chunk sub-piece instead of the whole 2.5MB group
                tiles = load_group(g, split=4 if g == NG - 1 else 1)
                mm_group(g, tiles)

            # ---- epilogue ----
            score_ps = psum.tile([1, NC_N], F32, tag="score", bufs=1, name="score")
            for dh in range(DH):
                t_sb = pool.tile([128, NC_N], F32, tag="t_sb", bufs=2, name="t_sb")
                nc.vector.tensor_tensor(t_sb[:], ea_ps[dh][:], ert_sb[:, dh, :], MUL)
                p_sb = pool.tile([128, NC_N], BF, tag="p_sb", bufs=2, name="p_sb")
                nc.vector.tensor_tensor(p_sb[:], eb_ps[dh][:], t_sb[:], MUL)
                nc.tensor.matmul(
                    score_ps[:],
                    ones_bf[:],
                    p_sb[:],
                    start=(dh == 0),
                    stop=(dh == DH - 1),
                )
            sig_sb = pool.tile([1, NC_N], F32, name="sig_sb")
            nc.scalar.activation(
                sig_sb[:], score_ps[:], mybir.ActivationFunctionType.Sigmoid
            )
            nc.sync.dma_start(out_dram[:].rearrange("n o -> o n"), sig_sb[:])

    nc.compile()
    return nc


def _get_nc():
    if "nc" not in _CACHE:
        _CACHE["nc"] = _build()
    return _CACHE["nc"]


def kernel(x, y, r, W_ent, W_rel):
    from concourse.bass_utils import run_bass_kernel_spmd

    x = np.asarray(x, dtype=np.float32)
    y = np.asarray(y, dtype=np.float32)
    r = np.asarray(r, dtype=np.float32)
    W_ent = np.asarray(W_ent, dtype=np.float32)
    W_rel = np.asarray(W_rel, dtype=np.float32)

    nc = _get_nc()

    wT_h = W_ent.T.astype(BF16NP, order="C")       # [E, D]
    wrT_h = W_rel.T.astype(BF16NP, order="C")      # [R, D]
    in_maps = []
    for c in range(NCORES):
        sl = slice(c * NC_N, (c + 1) * NC_N)
        in_maps.append(
            {
                "xT": x[sl].T.astype(BF16NP, order="C"),
                "yT": y[sl].T.astype(BF16NP, order="C"),
                "rT": r[sl].T.astype(BF16NP, order="C"),
                "wT": wT_h,
                "wrT": wrT_h,
            }
        )
    trace = bool(int(os.environ.get("KERNEL_TRACE", "0")))
    res = run_bass_kernel_spmd(
        nc, in_maps, core_ids=list(range(NCORES)), trace=trace
    )
    _CACHE["last_result"] = res
    out = np.concatenate([res.results[c]["out"] for c in range(NCORES)], axis=0)
    return out


# revision 11
# speedup vs baseline: 2.1130x; 1.0068x over previous
"""Trainium2 Bass kernel for nn_Compositional: sigmoid(sum(er*ea*eb, -1)).

  ea = x @ W_ent.T   [N, D]
  eb = y @ W_ent.T   [N, D]
  er = r @ W_rel.T   [N, D]
  out = sigmoid(sum_d er*ea*eb)  [N, 1]

Sharding: data-parallel over N across 8 cores (512 rows each), W_ent/W_rel
replicated.

Staging: all inputs are cast to bf16 and pre-transposed on the host so the
device streams [contraction, free] tiles directly (no PE transposes) at half
the HBM traffic of fp32.  Per-core DMA = xT 16MB + yT 16MB + wT 8MB + rT/wrT
0.75MB ~= 40.75MB (~113us at 360B/ns); PE = 2 GEMMs [512,16384]x[16384,256]
= 512 matmul instrs x 512 rows (~109us).  Memory/compute-balanced.

Per-core plan:
  - Everything computed transposed: eaT/ebT [D, n] with D on partitions.
  - Stream E in groups of 8 128-row chunks (w 0.5MB + x 1MB + y 1MB per
    group, double-buffered); per chunk, 4 accumulating matmuls
    (ea/eb x 2 d-halves) into persistent PSUM banks.
  - er phase after group 0 (small), stored to SBUF f32.
  - Epilogue: prod = eaT*ebT*erT on DVE, partition-reduce via ones-matmul,
    sigmoid on ACT, DMA out.
"""
import os

import numpy as np
import ml_dtypes

# Full-problem constants (hardcoded; kernel.py must be self-contained).
N, E, R, D = 4096, 16384, 512, 256
NCORES = 8
NC_N = N // NCORES      # 512 rows per core
GC = 8                  # 128-row e-chunks per DMA group
NCHUNK = E // 128       # 128 contraction chunks
NG = NCHUNK // GC       # 16 groups
DH = D // 128           # 2 d-halves
RC = R // 128           # 4 r-chunks

BF16NP = ml_dtypes.bfloat16

_CACHE = {}


def _build():
    import concourse.mybir as mybir
    import concourse.tile as tile
    from concourse import bacc

    F32 = mybir.dt.float32
    BF = mybir.dt.bfloat16
    MUL = mybir.AluOpType.mult

    nc = bacc.Bacc("TRN2", target_bir_lowering=False)

    xT_dram = nc.dram_tensor("xT", [E, NC_N], BF, kind="ExternalInput")
    yT_dram = nc.dram_tensor("yT", [E, NC_N], BF, kind="ExternalInput")
    rT_dram = nc.dram_tensor("rT", [R, NC_N], BF, kind="ExternalInput")
    wT_dram = nc.dram_tensor("wT", [E, D], BF, kind="ExternalInput")
    wrT_dram = nc.dram_tensor("wrT", [R, D], BF, kind="ExternalInput")
    out_dram = nc.dram_tensor("out", [NC_N, 1], F32, kind="ExternalOutput")

    with tile.TileContext(nc) as tc:
        with (
            tc.tile_pool(name="const", bufs=1) as cpool,
            tc.tile_pool(name="stream", bufs=1) as pool,
            tc.tile_pool(name="psum", bufs=1, space="PSUM") as psum,
        ):
            # ---- constants ----
            ones_bf = cpool.tile([128, 1], BF)
            nc.gpsimd.memset(ones_bf[:], 1.0)

            # erT for the epilogue products: dh0 kept f32 (multiplied against
            # PSUM f32), dh1 kept bf16 (fast all-bf16 DVE path)
            ert0_sb = cpool.tile([128, NC_N], F32)
            ert1_sb = cpool.tile([128, NC_N], BF)

            # ---- PSUM accumulators (persist through main loop) ----
            ea_ps = [
                psum.tile([128, NC_N], F32, tag=f"ea{dh}", bufs=1, name=f"ea{dh}")
                for dh in range(DH)
            ]
            eb_ps = [
                psum.tile([128, NC_N], F32, tag=f"eb{dh}", bufs=1, name=f"eb{dh}")
                for dh in range(DH)
            ]

            def load_group(g, pieces=None):
                """DMA one e-group (GC chunks) of w/x/y, optionally split into
                `pieces` (chunk counts) so dependent matmuls can start early."""
                w_nat = pool.tile([128, GC, D], BF, tag="w_nat", bufs=3, name="w_nat")
                x_nat = pool.tile([128, GC, NC_N], BF, tag="x_nat", bufs=3, name="x_nat")
                y_nat = pool.tile([128, GC, NC_N], BF, tag="y_nat", bufs=3, name="y_nat")
                pieces = pieces or [GC]
                assert sum(pieces) == GC
                c0 = 0
                for pc in pieces:
                    cs = slice(c0, c0 + pc)
                    rs = slice((g * GC + c0) * 128, (g * GC + c0 + pc) * 128)
                    nc.sync.dma_start(
                        w_nat[:, cs, :],
                        wT_dram[rs, :].rearrange("(c p) d -> p c d", p=128),
                    )
                    nc.sync.dma_start(
                        x_nat[:, cs, :],
                        xT_dram[rs, :].rearrange("(c p) n -> p c n", p=128),
                    )
                    nc.sync.dma_start(
                        y_nat[:, cs, :],
                        yT_dram[rs, :].rearrange("(c p) n -> p c n", p=128),
                    )
                    c0 += pc
                return w_nat, x_nat, y_nat

            def mm_group(g, tiles):
                w_nat, x_nat, y_nat = tiles
                for c in range(GC):
                    chunk = g * GC + c
                    start = chunk == 0
                    stop = chunk == NCHUNK - 1
                    if not stop:
                        for dh in range(DH):
                            wsl = w_nat[:, c, dh * 128 : (dh + 1) * 128]
                            nc.tensor.matmul(
                                ea_ps[dh][:], wsl, x_nat[:, c, :], start=start, stop=stop
                            )
                            nc.tensor.matmul(
                                eb_ps[dh][:], wsl, y_nat[:, c, :], start=start, stop=stop
                            )
                    else:
                        # final chunk: stagger the stops (ea0 first) so the
                        # epilogue product chains can start ASAP
                        for dh in range(DH):
                            nc.tensor.matmul(
                                ea_ps[dh][:],
                                w_nat[:, c, dh * 128 : (dh + 1) * 128],
                                x_nat[:, c, :],
                                start=start,
                                stop=stop,
                            )
                        for dh in range(DH):
                            nc.tensor.matmul(
                                eb_ps[dh][:],
                                w_nat[:, c, dh * 128 : (dh + 1) * 128],
                                y_nat[:, c, :],
                                start=start,
                                stop=stop,
                            )

            def rel_dma():
                wr_nat = pool.tile([128, RC, D], BF, tag="wr_nat", bufs=1, name="wr_nat")
                nc.sync.dma_start(
                    wr_nat[:], wrT_dram[:, :].rearrange("(c p) d -> p c d", p=128)
                )
                r_nat = pool.tile([128, RC, NC_N], BF, tag="r_nat", bufs=1, name="r_nat")
                nc.sync.dma_start(
                    r_nat[:], rT_dram[:, :].rearrange("(c p) n -> p c n", p=128)
                )
                return wr_nat, r_nat

            def rel_mm(tiles):
                wr_nat, r_nat = tiles
                er_ps = [
                    psum.tile([128, NC_N], F32, tag=f"er{dh}", bufs=1, name=f"er{dh}")
                    for dh in range(DH)
                ]
                for c in range(RC):
                    for dh in range(DH):
                        nc.tensor.matmul(
                            er_ps[dh][:],
                            wr_nat[:, c, dh * 128 : (dh + 1) * 128],
                            r_nat[:, c, :],
                            start=(c == 0),
                            stop=(c == RC - 1),
                        )
                nc.scalar.copy(ert0_sb[:], er_ps[0][:])
                nc.scalar.copy(ert1_sb[:], er_ps[1][:])

            # ---- main schedule ----
            g0 = load_group(0, pieces=[2, 2, 2, 2])
            rel = rel_dma()
            mm_group(0, g0)
            rel_mm(rel)
            for g in range(1, NG):
                # shrink the tail pieces so the final matmuls only wait on a
                # 1-chunk sub-piece instead of the whole 2.5MB group
                if g == NG - 1:
                    pieces = [4, 2, 1, 1]
                elif g == NG - 2:
                    pieces = [4, 4]
                else:
                    pieces = None
                tiles = load_group(g, pieces=pieces)
                mm_group(g, tiles)

            # ---- epilogue ----
            # dh0: straight from PSUM on DVE (f32).  dh1: ACT copies PSUM ->
            # bf16 SBUF in parallel, then all-bf16 DVE products run in the
            # 2x_1p fast mode.  The two chains overlap.
            score_ps = psum.tile([1, NC_N], F32, tag="score", bufs=1, name="score")

            ea1_bf = pool.tile([128, NC_N], BF, tag="cp_bf", bufs=2, name="ea1_bf")
            nc.scalar.copy(ea1_bf[:], ea_ps[1][:])
            t0_sb = pool.tile([128, NC_N], F32, tag="t_sb", bufs=2, name="t0_sb")
            nc.vector.tensor_tensor(t0_sb[:], ea_ps[0][:], ert0_sb[:], MUL)
            eb1_bf = pool.tile([128, NC_N], BF, tag="cp_bf", bufs=2, name="eb1_bf")
            nc.scalar.copy(eb1_bf[:], eb_ps[1][:])
            p0_sb = pool.tile([128, NC_N], BF, tag="p_sb", bufs=2, name="p0_sb")
            nc.vector.tensor_tensor(p0_sb[:], eb_ps[0][:], t0_sb[:], MUL)
            nc.tensor.matmul(score_ps[:], ones_bf[:], p0_sb[:], start=True, stop=False)
            t1_sb = pool.tile([128, NC_N], BF, tag="t_sb", bufs=2, name="t1_sb")
            nc.vector.tensor_tensor(t1_sb[:], ea1_bf[:], ert1_sb[:], MUL)
            p1_sb = pool.tile([128, NC_N], BF, tag="p_sb", bufs=2, name="p1_sb")
            nc.vector.tensor_tensor(p1_sb[:], eb1_bf[:], t1_sb[:], MUL)
            nc.tensor.matmul(score_ps[:], ones_bf[:], p1_sb[:], start=False, stop=True)
            sig_sb = pool.tile([1, NC_N], F32, name="sig_sb")
            nc.scalar.activation(
                sig_sb[:], score_ps[:], mybir.ActivationFunctionType.Sigmoid
            )
            nc.sync.dma_start(out_dram[:].rearrange("n o -> o n"), sig_sb[:])

    nc.compile()
    return nc


def _get_nc():
    if "nc" not in _CACHE:
        _CACHE["nc"] = _build()
    return _CACHE["nc"]


def kernel(x, y, r, W_ent, W_rel):
    from concourse.bass_utils import run_bass_kernel_spmd

    x = np.asarray(x, dtype=np.float32)
    y = np.asarray(y, dtype=np.float32)
    r = np.asarray(r, dtype=np.float32)
    W_ent = np.asarray(W_ent, dtype=np.float32)
    W_rel = np.asarray(W_rel, dtype=np.float32)

    nc = _get_nc()

    wT_h = W_ent.T.astype(BF16NP, order="C")       # [E, D]
    wrT_h = W_rel.T.astype(BF16NP, order="C")      # [R, D]
    in_maps = []
    for c in range(NCORES):
        sl = slice(c * NC_N, (c + 1) * NC_N)
        in_maps.append(
            {
                "xT": x[sl].T.astype(BF16NP, order="C"),
                "yT": y[sl].T.astype(BF16NP, order="C"),
                "rT": r[sl].T.astype(BF16NP, order="C"),
                "wT": wT_h,
                "wrT": wrT_h,
            }
        )
    trace = bool(int(os.environ.get("KERNEL_TRACE", "0")))
    res = run_bass_kernel_spmd(
        nc, in_maps, core_ids=list(range(NCORES)), trace=trace
    )
    _CACHE["last_result"] = res
    out = np.concatenate([res.results[c]["out"] for c in range(NCORES)], axis=0)
    return out


# revision 13
# speedup vs baseline: 2.1330x; 1.0095x over previous
"""Trainium2 Bass kernel for nn_Compositional: sigmoid(sum(er*ea*eb, -1)).

  ea = x @ W_ent.T   [N, D]
  eb = y @ W_ent.T   [N, D]
  er = r @ W_rel.T   [N, D]
  out = sigmoid(sum_d er*ea*eb)  [N, 1]

Sharding: data-parallel over N across 8 cores (512 rows each), W_ent/W_rel
replicated.

Staging: all inputs are cast to bf16 and pre-transposed on the host so the
device streams [contraction, free] tiles directly (no PE transposes) at half
the HBM traffic of fp32.  Per-core DMA = xT 16MB + yT 16MB + wT 8MB + rT/wrT
0.75MB ~= 40.75MB (~113us at 360B/ns); PE = 2 GEMMs [512,16384]x[16384,256]
= 512 matmul instrs x 512 rows (~109us).  Memory/compute-balanced.

Per-core plan:
  - Everything computed transposed: eaT/ebT [D, n] with D on partitions.
  - Stream E in groups of 8 128-row chunks (w 0.5MB + x 1MB + y 1MB per
    group, double-buffered); per chunk, 4 accumulating matmuls
    (ea/eb x 2 d-halves) into persistent PSUM banks.
  - er phase after group 0 (small), stored to SBUF f32.
  - Epilogue: prod = eaT*ebT*erT on DVE, partition-reduce via ones-matmul,
    sigmoid on ACT, DMA out.
"""
import os

import numpy as np
import ml_dtypes

# Full-problem constants (hardcoded; kernel.py must be self-contained).
N, E, R, D = 4096, 16384, 512, 256
NCORES = 8
NC_N = N // NCORES      # 512 rows per core
GC = 8                  # 128-row e-chunks per DMA group
NCHUNK = E // 128       # 128 contraction chunks
NG = NCHUNK // GC       # 16 groups
DH = D // 128           # 2 d-halves
RC = R // 128           # 4 r-chunks

BF16NP = ml_dtypes.bfloat16

_CACHE = {}


def _build():
    import concourse.mybir as mybir
    import concourse.tile as tile
    from concourse import bacc

    F32 = mybir.dt.float32
    BF = mybir.dt.bfloat16
    MUL = mybir.AluOpType.mult

    nc = bacc.Bacc("TRN2", target_bir_lowering=False)

    xT_dram = nc.dram_tensor("xT", [E, NC_N], BF, kind="ExternalInput")
    yT_dram = nc.dram_tensor("yT", [E, NC_N], BF, kind="ExternalInput")
    rT_dram = nc.dram_tensor("rT", [R, NC_N], BF, kind="ExternalInput")
    wT_dram = nc.dram_tensor("wT", [E, D], BF, kind="ExternalInput")
    wrT_dram = nc.dram_tensor("wrT", [R, D], BF, kind="ExternalInput")
    out_dram = nc.dram_tensor("out", [NC_N, 1], F32, kind="ExternalOutput")

    with tile.TileContext(nc) as tc:
        with (
            tc.tile_pool(name="const", bufs=1) as cpool,
            tc.tile_pool(name="stream", bufs=1) as pool,
            tc.tile_pool(name="psum", bufs=1, space="PSUM") as psum,
        ):
            # ---- constants ----
            ones_bf = cpool.tile([128, 1], BF)
            nc.gpsimd.memset(ones_bf[:], 1.0)

            # erT for the epilogue products: dh0 kept f32 (multiplied against
            # PSUM f32), dh1 kept bf16 (fast all-bf16 DVE path)
            ert0_sb = cpool.tile([128, NC_N], F32)
            ert1_sb = cpool.tile([128, NC_N], BF)

            # ---- PSUM accumulators (persist through main loop) ----
            ea_ps = [
                psum.tile([128, NC_N], F32, tag=f"ea{dh}", bufs=1, name=f"ea{dh}")
                for dh in range(DH)
            ]
            eb_ps = [
                psum.tile([128, NC_N], F32, tag=f"eb{dh}", bufs=1, name=f"eb{dh}")
                for dh in range(DH)
            ]

            def load_group(g, pieces=None):
                """DMA one e-group (GC chunks) of w/x/y, optionally split into
                `pieces` (chunk counts) so dependent matmuls can start early."""
                w_nat = pool.tile([128, GC, D], BF, tag="w_nat", bufs=3, name="w_nat")
                x_nat = pool.tile([128, GC, NC_N], BF, tag="x_nat", bufs=3, name="x_nat")
                y_nat = pool.tile([128, GC, NC_N], BF, tag="y_nat", bufs=3, name="y_nat")
                pieces = pieces or [GC]
                assert sum(pieces) == GC
                c0 = 0
                for pc in pieces:
                    cs = slice(c0, c0 + pc)
                    rs = slice((g * GC + c0) * 128, (g * GC + c0 + pc) * 128)
                    nc.sync.dma_start(
                        w_nat[:, cs, :],
                        wT_dram[rs, :].rearrange("(c p) d -> p c d", p=128),
                    )
                    nc.sync.dma_start(
                        x_nat[:, cs, :],
                        xT_dram[rs, :].rearrange("(c p) n -> p c n", p=128),
                    )
                    nc.sync.dma_start(
                        y_nat[:, cs, :],
                        yT_dram[rs, :].rearrange("(c p) n -> p c n", p=128),
                    )
                    c0 += pc
                return w_nat, x_nat, y_nat

            def mm_group(g, tiles, pieces=None):
                """Per piece: all ea matmuls first (need only w+x), then eb
                (needs y) — so PE starts before the piece's y DMA lands."""
                w_nat, x_nat, y_nat = tiles
                pieces = pieces or [GC]
                c0 = 0
                for pc in pieces:
                    for c in range(c0, c0 + pc):
                        chunk = g * GC + c
                        for dh in range(DH):
                            nc.tensor.matmul(
                                ea_ps[dh][:],
                                w_nat[:, c, dh * 128 : (dh + 1) * 128],
                                x_nat[:, c, :],
                                start=(chunk == 0),
                                stop=(chunk == NCHUNK - 1),
                            )
                    for c in range(c0, c0 + pc):
                        chunk = g * GC + c
                        for dh in range(DH):
                            nc.tensor.matmul(
                                eb_ps[dh][:],
                                w_nat[:, c, dh * 128 : (dh + 1) * 128],
                                y_nat[:, c, :],
                                start=(chunk == 0),
                                stop=(chunk == NCHUNK - 1),
                            )
                    c0 += pc

            def rel_dma():
                wr_nat = pool.tile([128, RC, D], BF, tag="wr_nat", bufs=1, name="wr_nat")
                nc.sync.dma_start(
                    wr_nat[:], wrT_dram[:, :].rearrange("(c p) d -> p c d", p=128)
                )
                r_nat = pool.tile([128, RC, NC_N], BF, tag="r_nat", bufs=1, name="r_nat")
                nc.sync.dma_start(
                    r_nat[:], rT_dram[:, :].rearrange("(c p) n -> p c n", p=128)
                )
                return wr_nat, r_nat

            def rel_mm(tiles):
                wr_nat, r_nat = tiles
                er_ps = [
                    psum.tile([128, NC_N], F32, tag=f"er{dh}", bufs=1, name=f"er{dh}")
                    for dh in range(DH)
                ]
                for c in range(RC):
                    for dh in range(DH):
                        nc.tensor.matmul(
                            er_ps[dh][:],
                            wr_nat[:, c, dh * 128 : (dh + 1) * 128],
                            r_nat[:, c, :],
                            start=(c == 0),
                            stop=(c == RC - 1),
                        )
                nc.scalar.copy(ert0_sb[:], er_ps[0][:])
                nc.scalar.copy(ert1_sb[:], er_ps[1][:])

            # ---- main schedule ----
            p0 = [2, 2, 2, 2]
            g0 = load_group(0, pieces=p0)
            rel = rel_dma()
            mm_group(0, g0, pieces=p0)
            rel_mm(rel)
            for g in range(1, NG):
                # every group in 2 pieces (halves PE's steady-state lag);
                # shrink the last group's tail so the final matmuls only
                # wait on a 1-chunk sub-piece
                pieces = [4, 2, 1, 1] if g == NG - 1 else [4, 4]
                tiles = load_group(g, pieces=pieces)
                mm_group(g, tiles, pieces=pieces)

            # ---- epilogue ----
            # dh0: straight from PSUM on DVE (f32).  dh1: ACT copies PSUM ->
            # bf16 SBUF in parallel, then all-bf16 DVE products run in the
            # 2x_1p fast mode.  The two chains overlap.
            score_ps = psum.tile([1, NC_N], F32, tag="score", bufs=1, name="score")

            ea1_bf = pool.tile([128, NC_N], BF, tag="cp_bf", bufs=2, name="ea1_bf")
            nc.scalar.copy(ea1_bf[:], ea_ps[1][:])
            t0_sb = pool.tile([128, NC_N], F32, tag="t_sb", bufs=2, name="t0_sb")
            nc.vector.tensor_tensor(t0_sb[:], ea_ps[0][:], ert0_sb[:], MUL)
            eb1_bf = pool.tile([128, NC_N], BF, tag="cp_bf", bufs=2, name="eb1_bf")
            nc.scalar.copy(eb1_bf[:], eb_ps[1][:])
            p0_sb = pool.tile([128, NC_N], BF, tag="p_sb", bufs=2, name="p0_sb")
            nc.vector.tensor_tensor(p0_sb[:], eb_ps[0][:], t0_sb[:], MUL)
            nc.tensor.matmul(score_ps[:], ones_bf[:], p0_sb[:], start=True, stop=False)
            t1_sb = pool.tile([128, NC_N], BF, tag="t_sb", bufs=2, name="t1_sb")
            nc.vector.tensor_tensor(t1_sb[:], ea1_bf[:], ert1_sb[:], MUL)
            p1_sb = pool.tile([128, NC_N], BF, tag="p_sb", bufs=2, name="p1_sb")
            nc.vector.tensor_tensor(p1_sb[:], eb1_bf[:], t1_sb[:], MUL)
            nc.tensor.matmul(score_ps[:], ones_bf[:], p1_sb[:], start=False, stop=True)
            sig_sb = pool.tile([1, NC_N], F32, name="sig_sb")
            nc.scalar.activation(
                sig_sb[:], score_ps[:], mybir.ActivationFunctionType.Sigmoid
            )
            nc.sync.dma_start(out_dram[:].rearrange("n o -> o n"), sig_sb[:])

    nc.compile()
    return nc


def _get_nc():
    if "nc" not in _CACHE:
        _CACHE["nc"] = _build()
    return _CACHE["nc"]


def kernel(x, y, r, W_ent, W_rel):
    from concourse.bass_utils import run_bass_kernel_spmd

    x = np.asarray(x, dtype=np.float32)
    y = np.asarray(y, dtype=np.float32)
    r = np.asarray(r, dtype=np.float32)
    W_ent = np.asarray(W_ent, dtype=np.float32)
    W_rel = np.asarray(W_rel, dtype=np.float32)

    nc = _get_nc()

    wT_h = W_ent.T.astype(BF16NP, order="C")       # [E, D]
    wrT_h = W_rel.T.astype(BF16NP, order="C")      # [R, D]
    in_maps = []
    for c in range(NCORES):
        sl = slice(c * NC_N, (c + 1) * NC_N)
        in_maps.append(
            {
                "xT": x[sl].T.astype(BF16NP, order="C"),
                "yT": y[sl].T.astype(BF16NP, order="C"),
                "rT": r[sl].T.astype(BF16NP, order="C"),
                "wT": wT_h,
                "wrT": wrT_h,
            }
        )
    trace = bool(int(os.environ.get("KERNEL_TRACE", "0")))
    res = run_bass_kernel_spmd(
        nc, in_maps, core_ids=list(range(NCORES)), trace=trace
    )
    _CACHE["last_result"] = res
    out = np.concatenate([res.results[c]["out"] for c in range(NCORES)], axis=0)
    return out
